# revision 12
# baseline (speedup 1.0000x reference)
"""Trainium2 Bass kernel for nn_DecodePredictions (YOLO-style decode, B=16).

Strategy: pure data-parallel over batch (2 images per core x 8 cores).
Host-side: concat the 3 prediction levels into a flat [N_anchor, 85] tensor
per image, pad 8400 -> 8448 anchors so everything divides evenly, and lay
anchors out partition-blocked so every DMA moves large contiguous
per-partition chunks. Score logits ship as fp8e4 (sigmoid rounding error
~1e-5 of absmax), box logits as fp32 planes; grid/stride constants are
precomputed host-side.

The whole output path is bf16: the gate is rel_err < 2e-2 against
absmax ~1958, and bf16 rounding of the box coords costs at most
ULP(2048)/2 = 4 absolute (~2e-3 relative), so halving the dominant
HBM write traffic is free accuracy-wise. The host upconverts to fp32
while scattering per-core results into the final array.

Device output layout is [anchor, lane, class] (lane-planes per anchor):
every SBUF write filling it is a contiguous run, which keeps DVE out of
its 0.3-elem/cycle scattered-write mode. The box planes (one value
repeated C times) are built in two stages: an int32 broadcast of
duplicated pairs into a 16-wide scratch, then a step-1 5x-replicating
copy into the output tile, so the expensive per-tile op runs in the
packed 2x/4x DVE modes. Host permutes [6,C]->[C,6] while upconverting.
"""

import ml_dtypes
import numpy as np

N_CORES = 8
B = 16
B_PER_CORE = B // N_CORES  # 2
C = 80
F = 85
N_REAL = 8400              # 80*80 + 40*40 + 20*20
N_PAD = 8448               # = 66 * 128
P = 128
KPP = B_PER_CORE * N_PAD // P  # 132 anchors per partition
GK = 11                    # anchors per partition per tile
NT = KPP // GK             # 12 tiles
NOB = 5                    # persistent output buffers
ICH = 33                   # anchors per input-chunk DMA (3 tiles)
NIC = KPP // ICH           # 4 input chunks

_CACHE: dict = {}


def _build_nc():
    import concourse.bacc as bacc
    import concourse.tile as tile
    from concourse import mybir
    from contextlib import ExitStack

    nc = bacc.Bacc("TRN2", target_bir_lowering=False, debug=False)
    pa01 = nc.dram_tensor("pa01", [P, KPP, 2], mybir.dt.float32, kind="ExternalInput")
    pa23 = nc.dram_tensor("pa23", [P, KPP, 2], mybir.dt.float32, kind="ExternalInput")
    auxS = nc.dram_tensor("auxS", [P, KPP, 2], mybir.dt.float32, kind="ExternalInput")
    auxB = nc.dram_tensor("auxB", [P, KPP, 2], mybir.dt.float32, kind="ExternalInput")
    predsB = nc.dram_tensor("predsB", [P, KPP, 81], mybir.dt.float8e4, kind="ExternalInput")
    clsc = nc.dram_tensor("clsc", [P, C], mybir.dt.bfloat16, kind="ExternalInput")
    out = nc.dram_tensor("out", [P, KPP, 6, C], mybir.dt.bfloat16, kind="ExternalOutput")

    fp32 = mybir.dt.float32
    bf16 = mybir.dt.bfloat16
    i32 = mybir.dt.int32
    AF = mybir.ActivationFunctionType

    with tile.TileContext(nc) as tc, ExitStack() as ctx:
        cpool = ctx.enter_context(tc.tile_pool(name="const", bufs=1))
        ipool = ctx.enter_context(tc.tile_pool(name="in", bufs=NIC))
        opool = ctx.enter_context(tc.tile_pool(name="out", bufs=1))
        tpool = ctx.enter_context(tc.tile_pool(name="tmp", bufs=NIC))

        # Consts on the Sync HWDGE ring (pa23 first: it gates the Exp that
        # gates everything); inputs + cls ride the second HWDGE ring (ACT)
        # so the two streams land concurrently.
        pa23_t = cpool.tile([P, KPP, 2], fp32, tag="pa23")
        nc.sync.dma_start(out=pa23_t[:], in_=pa23[:])
        pa01_t = cpool.tile([P, KPP, 2], fp32, tag="pa01")
        nc.sync.dma_start(out=pa01_t[:], in_=pa01[:])
        auxS_t = cpool.tile([P, KPP, 2], fp32, tag="auxS")
        nc.sync.dma_start(out=auxS_t[:], in_=auxS[:])
        auxB_t = cpool.tile([P, KPP, 2], fp32, tag="auxB")
        nc.sync.dma_start(out=auxB_t[:], in_=auxB[:])

        cls_t = cpool.tile([P, C], bf16, tag="cls")
        nc.scalar.dma_start(out=cls_t[:], in_=clsc[:])
        in_tiles = []
        for ci in range(NIC):
            it = ipool.tile([P, ICH, 81], mybir.dt.float8e4, tag="pt", name=f"pt{ci}")
            nc.scalar.dma_start(out=it[:], in_=predsB[:, ci * ICH : (ci + 1) * ICH, :])
            in_tiles.append(it)

        # Box decode in two halves (first 33 anchors, then the rest) so the
        # first output tile's chain is short. box_dup[p,k,l,d] duplicates
        # each corner value into an adjacent pair: (x1,x1),(y1,y1),(x2,x2),
        # (y2,y2) -- an int32 view then gives one register per repeated pair.
        wh_t = cpool.tile([P, KPP, 2], fp32, tag="wh")
        bb_t = cpool.tile([P, 2, KPP, 2], fp32, tag="bb")
        box_dup = cpool.tile([P, KPP, 4, 2], bf16, tag="boxd")
        nc.scalar.activation(wh_t[:], pa23_t[:], AF.Exp)
        nc.vector.tensor_mul(wh_t[:], wh_t[:], auxS_t[:])
        nc.vector.tensor_mul(bb_t[:, 0, :, :], pa01_t[:], auxS_t[:])
        nc.vector.tensor_add(bb_t[:, 0, :, :], bb_t[:, 0, :, :], auxB_t[:])
        nc.vector.tensor_add(bb_t[:, 1, :, :], bb_t[:, 0, :, :], wh_t[:])
        for jh in (0, 1):
            nc.vector.tensor_copy(
                box_dup[:, :, 2 * jh : 2 * jh + 2, :],
                bb_t[:, jh, :, :].unsqueeze(3).broadcast_to([P, KPP, 2, 2]),
            )

        # Persistent out buffers [anchor, lane, class]; constant class-id
        # plane (lane 4) written once per buffer on GpSimd (otherwise idle).
        ot_bufs = [
            opool.tile([P, GK, 6, C], bf16, tag=f"ot{j}", name=f"ot{j}")
            for j in range(NOB)
        ]
        for j in range(NOB):
            nc.gpsimd.tensor_copy(
                ot_bufs[j][:, :, 4, :],
                cls_t[:].unsqueeze(1).broadcast_to([P, GK, C]),
            )

        # Per chunk: sigmoid of the 80 class logits (contiguous), and
        # sigmoid of the objectness logit pre-broadcast across classes so
        # the per-tile score multiply is an all-step-1 TT (2x DVE mode).
        sig_cls, sig_obj = [], []
        for ci in range(NIC):
            sc = tpool.tile([P, ICH, 80], bf16, tag="sigc", name=f"sigc{ci}")
            nc.scalar.activation(sc[:], in_tiles[ci][:, :, 1:81], AF.Sigmoid)
            sig_cls.append(sc)
            so = tpool.tile([P, ICH, 80], bf16, tag="sigo", name=f"sigo{ci}")
            nc.scalar.activation(
                so[:], in_tiles[ci][:, :, 0:1].broadcast_to([P, ICH, 80]), AF.Sigmoid
            )
            sig_obj.append(so)

        for t in range(NT):
            sl = slice(t * GK, (t + 1) * GK)
            ci = t // 3
            ksl = slice((t % 3) * GK, (t % 3 + 1) * GK)
            ot = ot_bufs[t % NOB]

            # Stage 1: 8 copies of each duplicated int32 pair -> 16 repeats
            # of each bf16 corner value in scratch.
            rep = tpool.tile([P, GK, 4, 8], i32, tag="rep")
            nc.vector.tensor_copy(
                rep[:],
                box_dup[:, sl, :, :].bitcast(i32).broadcast_to([P, GK, 4, 8]),
            )
            # Stage 2: replicate the 16-wide runs 5x into the 80-wide box
            # lane planes -- src innermost is step-1 so DVE packs.
            nc.vector.tensor_copy(
                ot[:, :, 0:4, :].rearrange("p k l (r c) -> p k l r c", r=5),
                rep[:].bitcast(bf16).unsqueeze(3).broadcast_to([P, GK, 4, 5, 16]),
            )
            nc.vector.tensor_mul(
                ot[:, :, 5, :],
                sig_cls[ci][:, ksl, :],
                sig_obj[ci][:, ksl, :],
            )

            # Alternate the two HWDGE rings: each ring is FIFO per issuing
            # engine, and a DMA's ~2us completion receipt serializes with
            # the next DMA on the same ring. Two rings overlap that latency.
            dma_eng = nc.sync if t % 2 == 0 else nc.scalar
            dma_eng.dma_start(out=out[:, sl, :, :], in_=ot[:])

    nc.compile()
    return nc


def _host_consts():
    # Per-anchor (stride, stride) and (gx*stride, gy*stride), padded to N_PAD.
    s = np.ones(N_PAD, np.float32)
    bx = np.zeros(N_PAD, np.float32)
    by = np.zeros(N_PAD, np.float32)
    off = 0
    for g, st in ((80, 8.0), (40, 16.0), (20, 32.0)):
        n = g * g
        i = np.arange(n)
        s[off : off + n] = st
        bx[off : off + n] = (i % g) * st
        by[off : off + n] = (i // g) * st
        off += n
    auxS = np.stack([s, s], axis=-1).astype(np.float32)
    auxB = np.stack([bx, by], axis=-1).astype(np.float32)
    auxS = np.concatenate([auxS] * B_PER_CORE, 0).reshape(P, KPP, 2)
    auxB = np.concatenate([auxB] * B_PER_CORE, 0).reshape(P, KPP, 2)
    cls = np.broadcast_to(
        np.arange(C, dtype=np.float32).astype(ml_dtypes.bfloat16), (P, C)
    ).copy()
    return np.ascontiguousarray(auxS), np.ascontiguousarray(auxB), cls


def _host_in_maps(pred0, pred1, pred2):
    auxS, auxB, cls = _CACHE["consts"]
    pred0 = np.asarray(pred0, np.float32).reshape(B, -1, F)
    pred1 = np.asarray(pred1, np.float32).reshape(B, -1, F)
    pred2 = np.asarray(pred2, np.float32).reshape(B, -1, F)
    in_maps = []
    for core in range(N_CORES):
        flat = np.zeros((B_PER_CORE * N_PAD, F), np.float32)
        for j in range(B_PER_CORE):
            b = core * B_PER_CORE + j
            flat[j * N_PAD : j * N_PAD + N_REAL] = np.concatenate(
                [pred0[b], pred1[b], pred2[b]], axis=0
            )
        in_maps.append(
            {
                "pa01": np.ascontiguousarray(flat[:, 0:2]).reshape(P, KPP, 2),
                "pa23": np.ascontiguousarray(flat[:, 2:4]).reshape(P, KPP, 2),
                "auxS": auxS,
                "auxB": auxB,
                "predsB": np.ascontiguousarray(flat[:, 4:85])
                .astype(ml_dtypes.float8_e4m3fn)
                .reshape(P, KPP, 81),
                "clsc": cls,
            }
        )
    return in_maps


def kernel(images, pred0, pred1, pred2):
    from concourse.bass_utils import run_bass_kernel_spmd

    if "nc" not in _CACHE:
        _CACHE["consts"] = _host_consts()
        _CACHE["nc"] = _build_nc()
    nc = _CACHE["nc"]

    in_maps = _host_in_maps(pred0, pred1, pred2)
    res = run_bass_kernel_spmd(nc, in_maps, list(range(N_CORES)))
    final = np.empty((B, N_REAL * C, 6), np.float32)
    for core, r in enumerate(res.results):
        # Device layout is [anchor, lane, C]; upconvert bf16 -> fp32 on the
        # contiguous array first (vectorized), then swap to [anchor, C, lane]
        # with an fp32 strided assign -- orders of magnitude faster than one
        # fused strided bf16 cast-assign.
        f32 = r["out"].reshape(B_PER_CORE, N_PAD, 6, C)[:, :N_REAL].astype(
            np.float32
        )
        final[core * B_PER_CORE : (core + 1) * B_PER_CORE].reshape(
            B_PER_CORE, N_REAL, C, 6
        )[:] = f32.transpose(0, 1, 3, 2)
    return final


# revision 15
# speedup vs baseline: 1.0386x; 1.0386x over previous
"""Trainium2 Bass kernel for nn_DecodePredictions (YOLO-style decode, B=16).

Strategy: pure data-parallel over batch (2 images per core x 8 cores).
Host-side: concat the 3 prediction levels into a flat [N_anchor, 85] tensor
per image, pad 8400 -> 8448 anchors so everything divides evenly, and lay
anchors out partition-blocked so every DMA moves large contiguous
per-partition chunks. Score logits ship as fp8e4 (sigmoid rounding error
~1e-5 of absmax), box logits as fp32 planes; grid/stride constants are
precomputed host-side.

The whole output path is bf16: the gate is rel_err < 2e-2 against
absmax ~1958, and bf16 rounding of the box coords costs at most
ULP(2048)/2 = 4 absolute (~2e-3 relative), so halving the dominant
HBM write traffic is free accuracy-wise. The host upconverts to fp32
while scattering per-core results into the final array.

Device output layout is [anchor, lane, class] (lane-planes per anchor):
every SBUF write filling it is a contiguous run, which keeps DVE out of
its 0.3-elem/cycle scattered-write mode. The box planes (one value
repeated C times) are built in two stages: an int32 broadcast of
duplicated pairs into a 16-wide scratch, then a step-1 5x-replicating
copy into the output tile, so the expensive per-tile op runs in the
packed 2x/4x DVE modes. Host permutes [6,C]->[C,6] while upconverting.
"""

import ml_dtypes
import numpy as np

N_CORES = 8
B = 16
B_PER_CORE = B // N_CORES  # 2
C = 80
F = 85
N_REAL = 8400              # 80*80 + 40*40 + 20*20
N_PAD = 8448               # = 66 * 128
P = 128
KPP = B_PER_CORE * N_PAD // P  # 132 anchors per partition
GK = 11                    # anchors per partition per tile
NT = KPP // GK             # 12 tiles
NOB = 5                    # persistent output buffers
ICH = 33                   # anchors per input-chunk DMA (3 tiles)
NIC = KPP // ICH           # 4 input chunks

_CACHE: dict = {}


def _build_nc():
    import concourse.bacc as bacc
    import concourse.tile as tile
    from concourse import mybir
    from contextlib import ExitStack

    nc = bacc.Bacc("TRN2", target_bir_lowering=False, debug=False)
    pa01 = nc.dram_tensor("pa01", [P, KPP, 2], mybir.dt.float32, kind="ExternalInput")
    pa23 = nc.dram_tensor("pa23", [P, KPP, 2], mybir.dt.float32, kind="ExternalInput")
    auxS = nc.dram_tensor("auxS", [P, KPP, 2], mybir.dt.float32, kind="ExternalInput")
    auxB = nc.dram_tensor("auxB", [P, KPP, 2], mybir.dt.float32, kind="ExternalInput")
    predsB = nc.dram_tensor("predsB", [P, KPP, 81], mybir.dt.float8e4, kind="ExternalInput")
    clsc = nc.dram_tensor("clsc", [P, C], mybir.dt.bfloat16, kind="ExternalInput")
    out = nc.dram_tensor("out", [P, KPP, 6, C], mybir.dt.bfloat16, kind="ExternalOutput")

    fp32 = mybir.dt.float32
    bf16 = mybir.dt.bfloat16
    i32 = mybir.dt.int32
    AF = mybir.ActivationFunctionType

    with tile.TileContext(nc) as tc, ExitStack() as ctx:
        cpool = ctx.enter_context(tc.tile_pool(name="const", bufs=1))
        ipool = ctx.enter_context(tc.tile_pool(name="in", bufs=NIC))
        opool = ctx.enter_context(tc.tile_pool(name="out", bufs=1))
        tpool = ctx.enter_context(tc.tile_pool(name="tmp", bufs=NIC))

        # Consts on the Sync HWDGE ring (pa23 first: it gates the Exp that
        # gates everything); inputs + cls ride the second HWDGE ring (ACT)
        # so the two streams land concurrently.
        pa23_t = cpool.tile([P, KPP, 2], fp32, tag="pa23")
        nc.sync.dma_start(out=pa23_t[:], in_=pa23[:])
        pa01_t = cpool.tile([P, KPP, 2], fp32, tag="pa01")
        nc.sync.dma_start(out=pa01_t[:], in_=pa01[:])
        auxS_t = cpool.tile([P, KPP, 2], fp32, tag="auxS")
        nc.sync.dma_start(out=auxS_t[:], in_=auxS[:])
        auxB_t = cpool.tile([P, KPP, 2], fp32, tag="auxB")
        nc.sync.dma_start(out=auxB_t[:], in_=auxB[:])

        cls_t = cpool.tile([P, C], bf16, tag="cls")
        nc.scalar.dma_start(out=cls_t[:], in_=clsc[:])
        # Input chunks go on the SAME sync ring, behind the consts: the
        # SDMA engines round-robin rings at packet granularity, so putting
        # these on the other ring delays the consts (and the box decode that
        # gates everything) by ~5us. FIFO behind the consts they still land
        # before the sigmoids need them.
        in_tiles = []
        for ci in range(NIC):
            it = ipool.tile([P, ICH, 81], mybir.dt.float8e4, tag="pt", name=f"pt{ci}")
            nc.sync.dma_start(out=it[:], in_=predsB[:, ci * ICH : (ci + 1) * ICH, :])
            in_tiles.append(it)

        # Box decode in two halves (first 33 anchors, then the rest) so the
        # first output tile's chain is short. box_dup[p,k,l,d] duplicates
        # each corner value into an adjacent pair: (x1,x1),(y1,y1),(x2,x2),
        # (y2,y2) -- an int32 view then gives one register per repeated pair.
        wh_t = cpool.tile([P, KPP, 2], fp32, tag="wh")
        bb_t = cpool.tile([P, 2, KPP, 2], fp32, tag="bb")
        box_dup = cpool.tile([P, KPP, 4, 2], bf16, tag="boxd")
        nc.scalar.activation(wh_t[:], pa23_t[:], AF.Exp)
        nc.vector.tensor_mul(wh_t[:], wh_t[:], auxS_t[:])
        nc.vector.tensor_mul(bb_t[:, 0, :, :], pa01_t[:], auxS_t[:])
        nc.vector.tensor_add(bb_t[:, 0, :, :], bb_t[:, 0, :, :], auxB_t[:])
        nc.vector.tensor_add(bb_t[:, 1, :, :], bb_t[:, 0, :, :], wh_t[:])
        for jh in (0, 1):
            nc.vector.tensor_copy(
                box_dup[:, :, 2 * jh : 2 * jh + 2, :],
                bb_t[:, jh, :, :].unsqueeze(3).broadcast_to([P, KPP, 2, 2]),
            )

        # Persistent out buffers [anchor, lane, class]; constant class-id
        # plane (lane 4) written once per buffer on GpSimd (otherwise idle).
        ot_bufs = [
            opool.tile([P, GK, 6, C], bf16, tag=f"ot{j}", name=f"ot{j}")
            for j in range(NOB)
        ]
        for j in range(NOB):
            nc.gpsimd.tensor_copy(
                ot_bufs[j][:, :, 4, :],
                cls_t[:].unsqueeze(1).broadcast_to([P, GK, C]),
            )

        # Per chunk: sigmoid of the 80 class logits (contiguous), and
        # sigmoid of the objectness logit pre-broadcast across classes so
        # the per-tile score multiply is an all-step-1 TT (2x DVE mode).
        # Tile-0-sized sigmoid pair first (~1us each instead of 2.4) so the
        # first score TT unblocks as soon as the box path is ready.
        sc_t0 = tpool.tile([P, GK, 80], bf16, tag="sigct0")
        nc.scalar.activation(sc_t0[:], in_tiles[0][:, 0:GK, 1:81], AF.Sigmoid)
        so_t0 = tpool.tile([P, GK, 80], bf16, tag="sigot0")
        nc.scalar.activation(
            so_t0[:], in_tiles[0][:, 0:GK, 0:1].broadcast_to([P, GK, 80]), AF.Sigmoid
        )
        sig_cls, sig_obj = [], []
        for ci in range(NIC):
            sc = tpool.tile([P, ICH, 80], bf16, tag="sigc", name=f"sigc{ci}")
            nc.scalar.activation(sc[:], in_tiles[ci][:, :, 1:81], AF.Sigmoid)
            sig_cls.append(sc)
            so = tpool.tile([P, ICH, 80], bf16, tag="sigo", name=f"sigo{ci}")
            nc.scalar.activation(
                so[:], in_tiles[ci][:, :, 0:1].broadcast_to([P, ICH, 80]), AF.Sigmoid
            )
            sig_obj.append(so)

        for t in range(NT):
            sl = slice(t * GK, (t + 1) * GK)
            ci = t // 3
            ksl = slice((t % 3) * GK, (t % 3 + 1) * GK)
            ot = ot_bufs[t % NOB]

            # Stage 1: 8 copies of each duplicated int32 pair -> 16 repeats
            # of each bf16 corner value in scratch.
            rep = tpool.tile([P, GK, 4, 8], i32, tag="rep")
            nc.vector.tensor_copy(
                rep[:],
                box_dup[:, sl, :, :].bitcast(i32).broadcast_to([P, GK, 4, 8]),
            )
            # Stage 2: replicate the 16-wide runs 5x into the 80-wide box
            # lane planes -- src innermost is step-1 so DVE packs.
            nc.vector.tensor_copy(
                ot[:, :, 0:4, :].rearrange("p k l (r c) -> p k l r c", r=5),
                rep[:].bitcast(bf16).unsqueeze(3).broadcast_to([P, GK, 4, 5, 16]),
            )
            if t == 0:
                nc.vector.tensor_mul(ot[:, :, 5, :], sc_t0[:], so_t0[:])
            else:
                nc.vector.tensor_mul(
                    ot[:, :, 5, :],
                    sig_cls[ci][:, ksl, :],
                    sig_obj[ci][:, ksl, :],
                )

            nc.sync.dma_start(out=out[:, sl, :, :], in_=ot[:])

    nc.compile()
    return nc


def _host_consts():
    # Per-anchor (stride, stride) and (gx*stride, gy*stride), padded to N_PAD.
    s = np.ones(N_PAD, np.float32)
    bx = np.zeros(N_PAD, np.float32)
    by = np.zeros(N_PAD, np.float32)
    off = 0
    for g, st in ((80, 8.0), (40, 16.0), (20, 32.0)):
        n = g * g
        i = np.arange(n)
        s[off : off + n] = st
        bx[off : off + n] = (i % g) * st
        by[off : off + n] = (i // g) * st
        off += n
    auxS = np.stack([s, s], axis=-1).astype(np.float32)
    auxB = np.stack([bx, by], axis=-1).astype(np.float32)
    auxS = np.concatenate([auxS] * B_PER_CORE, 0).reshape(P, KPP, 2)
    auxB = np.concatenate([auxB] * B_PER_CORE, 0).reshape(P, KPP, 2)
    cls = np.broadcast_to(
        np.arange(C, dtype=np.float32).astype(ml_dtypes.bfloat16), (P, C)
    ).copy()
    return np.ascontiguousarray(auxS), np.ascontiguousarray(auxB), cls


def _host_in_maps(pred0, pred1, pred2):
    auxS, auxB, cls = _CACHE["consts"]
    pred0 = np.asarray(pred0, np.float32).reshape(B, -1, F)
    pred1 = np.asarray(pred1, np.float32).reshape(B, -1, F)
    pred2 = np.asarray(pred2, np.float32).reshape(B, -1, F)
    in_maps = []
    for core in range(N_CORES):
        flat = np.zeros((B_PER_CORE * N_PAD, F), np.float32)
        for j in range(B_PER_CORE):
            b = core * B_PER_CORE + j
            flat[j * N_PAD : j * N_PAD + N_REAL] = np.concatenate(
                [pred0[b], pred1[b], pred2[b]], axis=0
            )
        in_maps.append(
            {
                "pa01": np.ascontiguousarray(flat[:, 0:2]).reshape(P, KPP, 2),
                "pa23": np.ascontiguousarray(flat[:, 2:4]).reshape(P, KPP, 2),
                "auxS": auxS,
                "auxB": auxB,
                "predsB": np.ascontiguousarray(flat[:, 4:85])
                .astype(ml_dtypes.float8_e4m3fn)
                .reshape(P, KPP, 81),
                "clsc": cls,
            }
        )
    return in_maps


def kernel(images, pred0, pred1, pred2):
    from concourse.bass_utils import run_bass_kernel_spmd

    if "nc" not in _CACHE:
        _CACHE["consts"] = _host_consts()
        _CACHE["nc"] = _build_nc()
    nc = _CACHE["nc"]

    in_maps = _host_in_maps(pred0, pred1, pred2)
    res = run_bass_kernel_spmd(nc, in_maps, list(range(N_CORES)))
    final = np.empty((B, N_REAL * C, 6), np.float32)
    for core, r in enumerate(res.results):
        # Device layout is [anchor, lane, C]; upconvert bf16 -> fp32 on the
        # contiguous array first (vectorized), then swap to [anchor, C, lane]
        # with an fp32 strided assign -- orders of magnitude faster than one
        # fused strided bf16 cast-assign.
        f32 = r["out"].reshape(B_PER_CORE, N_PAD, 6, C)[:, :N_REAL].astype(
            np.float32
        )
        final[core * B_PER_CORE : (core + 1) * B_PER_CORE].reshape(
            B_PER_CORE, N_REAL, C, 6
        )[:] = f32.transpose(0, 1, 3, 2)
    return final
